# revision 13
# baseline (speedup 1.0000x reference)
"""Trainium2 Bass kernel for LocalConnect (locally-connected conv 3x3, pad 1).

Problem (hardcoded):
  x:      [B=32, C=64, 32, 32] f32
  weight: [1, O=64, C=64, 32, 32, 9] f32  (unshared per-position weights)
  out:    [B=32, O=64, 32, 32] f32
  out[b,o,y,x] = sum_{c,k} xpad[b,c,y+ky-1,x+kx-1] * w[o,c,y,x,k],  k=(ky,kx)

Strategy: spatial sharding over output rows — each of the 8 cores owns 4
output rows and the full batch/out_ch. Per-position weights are the dominant
traffic (151 MiB total, ~19 MiB/core) and are re-laid-out host-side so every
DMA is multi-KiB-contiguous.

Per output position the compute is a tiny matmul over the contraction
(c,k) = 576.  It is split into 6 chunks: 3 tap-PAIRS (taps (0,1),(3,4),(6,7),
K=128 = 2 taps x 64 c) and 3 tap SINGLES (taps 2,5,8, K=64).  The input
x-slab is stored twice in SBUF (partitions 0-63 and 64-127) with the second
copy offset by one element, so a tap-pair patch operand is a pure AP view of
the slab: partitions 0-63 read tap dx=0 and partitions 64-127 read tap dx=1
at the same within-partition address.  No patch materialization is ever done.

Variant "f32A": weights stationary (lhsT [K, O=64]), patches moving (N=B=32).
Variant "b16B": bf16 operands, patches stationary (lhsT [K, B=32]), weights
moving (N=O=64); fp32 PSUM accumulation.
"""

import os
import sys

import numpy as np

for _p in ("/opt/trn_rl_repo",):
    if os.path.isdir(_p) and _p not in sys.path:
        sys.path.append(_p)

import concourse.bass as bass
import concourse.mybir as mybir
from concourse import bacc
from concourse.tile import TileContext
from concourse.bass_utils import run_bass_kernel_spmd

# ---------------------------------------------------------------- constants
B, C, O = 32, 64, 64
H = W = 32
NCORES = 8
YPC = H // NCORES          # output rows per core = 4
HS = YPC + 2               # slab rows (with halo) = 6
WS = W + 2                 # slab cols (with pad)  = 34
PAIR_BASE = (0, 3, 6)      # tap pairs (k, k+1): dy = k // 3, dx = 0 & 1
SINGLES = (2, 5, 8)        # dy = k // 3, dx = 2
GP = 16                    # positions per group (one PSUM bank @ f32A)
NPOS = YPC * W             # positions per core = 128
NG = NPOS // GP            # groups per core = 8
SLAB_N = HS * B * WS       # slab payload elems per partition = 6528
SLAB_ALLOC = SLAB_N + 64   # tail pad so rearrange windows stay in bounds

VARIANT = os.environ.get("LC_VARIANT", "f32A")

_CACHE = {}


def _patch_view(slab, kparts, h, w):
    """AP [kparts, B] over the slab: element (p, b) = slab col 1+(h*B+b)*WS+w.

    For kparts=128 the upper 64 partitions hold the one-element-shifted slab
    copy, so they read tap dx+1 at the same address.
    """
    off = 1 + h * B * WS + w
    return slab[0:kparts, off : off + B * WS].rearrange(
        "p (b w) -> p b w", b=B, w=WS
    )[:, :, 0:1]


def _build_nc(variant, bench_iters=1):
    from contextlib import nullcontext

    f32 = mybir.dt.float32
    cdt = f32 if variant == "f32A" else mybir.dt.bfloat16
    nc = bacc.Bacc("TRN2", target_bir_lowering=False, debug=False)

    xslab = nc.dram_tensor("xslab", [128, SLAB_ALLOC], cdt, kind="ExternalInput")
    wp = nc.dram_tensor("wp", [3, 2 * C, NPOS, O], cdt, kind="ExternalInput")
    ws = nc.dram_tensor("ws", [3, C, NPOS, O], cdt, kind="ExternalInput")
    if variant == "f32A":
        out = nc.dram_tensor("out", [YPC, O, B, W], f32, kind="ExternalOutput")
    else:
        out = nc.dram_tensor("out", [YPC, B, O, W], f32, kind="ExternalOutput")

    with TileContext(nc) as tc:
        with (
            tc.tile_pool(name="slabp", bufs=1) as slabp,
            tc.tile_pool(name="wpp", bufs=3) as wpp,
            tc.tile_pool(name="wsp", bufs=3) as wsp,
            tc.tile_pool(name="stp", bufs=2) as stp,
            tc.tile_pool(
                name="psp", bufs=4 if variant == "f32A" else 3, space="PSUM"
            ) as psp,
            tc.tile_pool(name="scp", bufs=1, space="PSUM") as scp,
            tc.For_i(0, bench_iters, 1) if bench_iters > 1 else nullcontext(),
        ):
            slab = slabp.tile([128, SLAB_ALLOC], cdt)
            nc.sync.dma_start(out=slab[:, :], in_=xslab[:, :])
            # Dummy matmul: absorbs the slab-DMA wait on the PE engine so
            # later matmuls don't exceed walrus's per-instruction sync-wait
            # budget (they then only wait on their weight DMA + psum slot).
            scratch = scp.tile([32, 32], f32)
            nc.tensor.matmul(
                scratch[:, :], slab[0:128, 0:32], slab[0:128, 0:32],
                start=True, stop=True,
            )

            for y in range(YPC):
                if variant == "f32A":
                    stage = stp.tile([O, B * W], f32)
                else:
                    stage = stp.tile([B, O * W], f32)
                for half in range(NG // YPC):
                    p0 = (y * (NG // YPC) + half) * GP
                    xs = half * GP
                    wpt = wpp.tile([2 * C, 3 * GP * O], cdt)
                    nc.sync.dma_start(
                        out=wpt[:, :].rearrange(
                            "k (m p o) -> k m p o", m=3, p=GP, o=O
                        ),
                        in_=wp[:, :, p0 : p0 + GP, :].rearrange(
                            "m k p o -> k m p o"
                        ),
                    )
                    wst = wsp.tile([C, 3 * GP * O], cdt)
                    nc.sync.dma_start(
                        out=wst[:, :].rearrange(
                            "k (m p o) -> k m p o", m=3, p=GP, o=O
                        ),
                        in_=ws[:, :, p0 : p0 + GP, :].rearrange(
                            "m k p o -> k m p o"
                        ),
                    )
                    if variant == "f32A":
                        ps = psp.tile([O, GP * B], f32)
                    else:
                        ps = psp.tile([B, GP * O], f32)
                    for p in range(GP):
                        x = xs + p
                        if variant == "f32A":
                            outap = ps[:, p * B : (p + 1) * B]
                        else:
                            outap = ps[:, p * O : (p + 1) * O]
                        for j, k0 in enumerate(PAIR_BASE):
                            wap = wpt[:, (j * GP + p) * O : (j * GP + p + 1) * O]
                            pap = _patch_view(slab, 2 * C, y + k0 // 3, x)
                            if variant == "f32A":
                                nc.tensor.matmul(
                                    outap, wap, pap, start=(j == 0), stop=False
                                )
                            else:
                                nc.tensor.matmul(
                                    outap, pap, wap, start=(j == 0), stop=False
                                )
                        for i, k1 in enumerate(SINGLES):
                            wap = wst[:, (i * GP + p) * O : (i * GP + p + 1) * O]
                            pap = _patch_view(slab, C, y + k1 // 3, x + 2)
                            if variant == "f32A":
                                nc.tensor.matmul(
                                    outap, wap, pap, start=False, stop=(i == 2)
                                )
                            else:
                                nc.tensor.matmul(
                                    outap, pap, wap, start=False, stop=(i == 2)
                                )
                    if variant == "f32A":
                        src = ps[:, :].rearrange("o (p b) -> o p b", p=GP, b=B)
                        dst = stage[:, :].rearrange(
                            "o (b x) -> o x b", b=B, x=W
                        )[:, xs : xs + GP, :]
                    else:
                        src = ps[:, :].rearrange("b (p o) -> b p o", p=GP, o=O)
                        dst = stage[:, :].rearrange(
                            "b (o x) -> b x o", o=O, x=W
                        )[:, xs : xs + GP, :]
                    nc.vector.tensor_copy(out=dst, in_=src)
                nc.sync.dma_start(
                    out=out[y : y + 1].rearrange("one a b x -> (one a) (b x)"),
                    in_=stage[:, :],
                )
    nc.finalize()
    return nc


def _prep_inputs(x, weight, variant):
    """Host-side shard + re-layout. Returns per-core in_maps."""
    x = np.asarray(x, dtype=np.float32)
    weight = np.asarray(weight, dtype=np.float32)
    if variant == "f32A":
        cdt = np.float32
    else:
        import ml_dtypes

        cdt = ml_dtypes.bfloat16

    xpad = np.pad(x, ((0, 0), (0, 0), (1, 1), (1, 1)))  # [B, C, 34, 34]
    # wk[k, c, y, x, o]
    wk = np.ascontiguousarray(weight[0].transpose(4, 1, 2, 3, 0))
    in_maps = []
    for core in range(NCORES):
        y0 = core * YPC
        base = (
            xpad[:, :, y0 : y0 + HS, :]
            .transpose(1, 2, 0, 3)
            .reshape(C, SLAB_N)
            .astype(cdt)
        )
        # Doubled slab image: partitions 0-63 hold the slab at col offset 1,
        # partitions 64-127 the same data at col offset 0, so a single
        # [128, n] AP view reads tap dx on the lower half and tap dx+1 on
        # the upper half at the same within-partition address.
        slab = np.zeros((128, SLAB_ALLOC), dtype=cdt)
        slab[0:C, 1 : 1 + SLAB_N] = base
        slab[C:, 0:SLAB_N] = base
        wkc = wk[:, :, y0 : y0 + YPC, :, :].reshape(9, C, NPOS, O)
        wp_arr = np.stack(
            [np.concatenate([wkc[k], wkc[k + 1]], axis=0) for k in PAIR_BASE]
        ).astype(cdt)
        ws_arr = np.stack([wkc[k] for k in SINGLES]).astype(cdt)
        in_maps.append(
            {
                "xslab": np.ascontiguousarray(slab),
                "wp": np.ascontiguousarray(wp_arr),
                "ws": np.ascontiguousarray(ws_arr),
            }
        )
    return in_maps


def _assemble(results, variant):
    full = np.empty((B, O, H, W), dtype=np.float32)
    for core in range(NCORES):
        y0 = core * YPC
        r = results[core]["out"]
        if variant == "f32A":  # r: [YPC, O, B, W]
            full[:, :, y0 : y0 + YPC, :] = r.transpose(2, 1, 0, 3)
        else:  # r: [YPC, B, O, W]
            full[:, :, y0 : y0 + YPC, :] = r.transpose(1, 2, 0, 3)
    return full


def _get_nc(variant, bench_iters=1):
    key = (variant, bench_iters)
    if key not in _CACHE:
        _CACHE[key] = _build_nc(variant, bench_iters)
    return _CACHE[key]


def run(x, weight, variant=None, trace=False, bench_iters=1, **kw):
    variant = variant or VARIANT
    nc = _get_nc(variant, bench_iters)
    in_maps = _prep_inputs(x, weight, variant)
    res = run_bass_kernel_spmd(
        nc, in_maps, core_ids=list(range(NCORES)), trace=trace, **kw
    )
    return _assemble(res.results, variant), res


def kernel(x, weight):
    out, _ = run(x, weight)
    return out
